# revision 42
# baseline (speedup 1.0000x reference)
"""Causal self-attention (B=4, S=2048, D=1024, H=16) on 8 TRN2 NeuronCores.

Sharding (tensor-parallel on heads + data-parallel on batch):
  core c -> batch c//2, head-half c%2 (8 of 16 heads).
  Wq/Wk/Wv column-split, Wo row-split; the two partial outputs per batch are
  summed on the host (+ bo), which is the row-parallel unshard.

Per-core Bass/Tile program (matmul operands bf16, psum/softmax fp32),
structured as a superblock pipeline so every engine always has ready work
(Tile's list scheduler dispatches ready instructions by emission priority):

  per 512-query superblock i:
    q/k projections for superblock i only (feature-major; bias+cast on DVE);
    superblock 0 runs 6 concurrent kt-accumulation chains (borrowing the
      idle qk psum slots) so the PE advances with each arriving DMA tile
      instead of stalling one chain per wqk[kt];
    v projection (token-major, per-head ones column produces sumexp in the
      PV matmul), emitted mid-attention as PE filler;
    attention per head pair: scores via two concurrent row-group matmuls
      into a combined [128, 2, 512] psum tile; ONE exp activation covers
      both heads (halves the 352-cycle ACT call overhead); the causal mask
      is applied AFTER exp as a multiplicative 0/1 bf16 mask on probs
      (off the scores->exp critical chain, 4x DVE mode);
    probs persist in SBUF so the two heads run as separate single-bank PV
      accumulations: head 0 pipelined one key tile behind exp, head 1 as a
      dense deferred stream that fires after the next pair's first exps are
      queued (fills PE while ACT works);
    normalization: sumexp row broadcast by a K=1 bf16 matmul at array row
      64, single-op fast reciprocal on the broadcast (base partition 0),
      DVE multiplies; pv psum released early via copies to SBUF;
    output projection per superblock, emitted as PE filler for the next
      superblock's attention.

461.7us baseline -> ~323us, rel err 3.8e-3 (tolerance 2e-2).
"""

from contextlib import ExitStack

import numpy as np
import ml_dtypes

import concourse.bass as bass
import concourse.bacc as bacc
import concourse.tile as tile
import concourse.mybir as mybir

F32 = mybir.dt.float32
F32R = mybir.dt.float32r
BF16 = mybir.dt.bfloat16

def build_core_program(S=2048, D=1024, HC=8, DH=64, SQ=512, mm_dt=BF16,
                       probs_bufs=17):
    """Build the per-core Bass program (SPMD: same program, different data).
    Host passes xT/wqk/wv/wo as bfloat16; 0.125 q-scale folded into Wq/bq."""
    DQ = HC * DH              # head-slice width (512)
    DK = D // 128             # contraction tiles for projections (8)
    DQN = DQ // 128           # head-pair tiles (4)
    NSB = S // SQ             # query superblocks (4)
    NTT = S // 128            # token tiles (16)
    NOUT = min(512, D)        # output-proj free width
    NOB = D // NOUT           # output-proj col blocks (2)
    ND = SQ // 128            # 128-token tiles per superblock (4)
    assert DQ % 128 == 0 and S % SQ == 0 and SQ % 128 == 0 and D % 128 == 0

    bf = mm_dt == BF16
    in_dt = BF16 if bf else F32

    def m(ap):
        return ap if bf else ap.bitcast(F32R)

    nc = bacc.Bacc("TRN2", target_bir_lowering=False, debug=False)

    xT = nc.dram_tensor("xT", [D, S], in_dt, kind="ExternalInput").ap()
    wqk = nc.dram_tensor("wqk", [D, 2 * DQ], in_dt, kind="ExternalInput").ap()
    wv = nc.dram_tensor("wv", [D, DQ], in_dt, kind="ExternalInput").ap()
    wo = nc.dram_tensor("wo", [DQ, D], in_dt, kind="ExternalInput").ap()
    bqk = nc.dram_tensor("bqk", [2 * DQ], F32, kind="ExternalInput").ap()
    bv = nc.dram_tensor("bv", [DQ], F32, kind="ExternalInput").ap()
    out = nc.dram_tensor("out", [S, D], F32, kind="ExternalOutput").ap()

    with tile.TileContext(nc) as tc, ExitStack() as ctx:
        ctx.enter_context(nc.allow_low_precision(
            reason="low-precision matmul operands; accumulation stays fp32"))
        const = ctx.enter_context(tc.tile_pool(name="const", bufs=1))
        big = ctx.enter_context(tc.tile_pool(name="big", bufs=1))
        stream = ctx.enter_context(tc.tile_pool(name="stream", bufs=1))
        psum = ctx.enter_context(tc.tile_pool(name="psum", bufs=1, space="PSUM"))

        # ---- constants ----
        # 0/1 causal mask for the diagonal boundary subtile (bf16, applied
        # multiplicatively to probs AFTER exp, off the scores->exp chain)
        tri_f = const.tile([128, 128], F32)
        nc.vector.memset(tri_f[:], 1.0)
        nc.gpsimd.affine_select(
            out=tri_f[:], in_=tri_f[:], compare_op=mybir.AluOpType.is_ge,
            fill=0.0, base=0, channel_multiplier=-1, pattern=[[1, 128]],
        )
        tri01 = const.tile([128, 128], BF16)
        nc.vector.tensor_copy(tri01[:], tri_f[:])
        ones_hc = const.tile([128, HC], F32)
        nc.vector.memset(ones_hc[:], 1.0)
        # ones row living on partition 64 (for K=1 matmuls whose moving
        # operand is the partition-64 sumexp row)
        ones_hi = const.tile([65, 64], BF16)
        nc.vector.memset(ones_hi[64:65, :], 1.0)

        # biases: bqk as [128, 2*DQN] (column t = dout tile t), bv broadcast
        bqk_sb = const.tile([128, 2 * DQN], F32)
        nc.sync.dma_start(bqk_sb[:], bqk.rearrange("(t p) -> p t", p=128))
        bv_rowf = const.tile([1, DQ], F32)
        nc.sync.dma_start(bv_rowf[:], bv.rearrange("(a d) -> a d", a=1))
        bv_row = const.tile([1, DQ], BF16)
        nc.vector.tensor_copy(bv_row[:], bv_rowf[:])
        ones1b = const.tile([1, 128], BF16)
        nc.vector.memset(ones1b[:], 1.0)
        bv_bc = const.tile([128, DQ], F32)

        # ---- big resident tensors ----
        kT = big.tile([128, DQN, S], mm_dt)     # [pair 2x64 rows, tokens]
        qT = big.tile([128, DQN, S], mm_dt)
        v_aug = big.tile([128, NTT, HC * 65], mm_dt)
        wqk_sb = big.tile([128, DK, 2 * DQ], mm_dt)
        wv_sb = big.tile([128, DK, DQ], mm_dt)
        wo_sb = big.tile([128, DQN, D], mm_dt)
        xt_all = big.tile([128, DK, S], mm_dt)

        # DMA priority order: wqk -> xt(sb0) -> wv -> xt(sb1..3) -> wo,
        # so superblock-0 projections and attention can start early
        for kt in range(DK):
            # alternate issue queues so the startup-critical wqk tiles
            # stream on two DMA queues in parallel
            eng = nc.gpsimd if kt % 2 == 0 else nc.scalar
            eng.dma_start(wqk_sb[:, kt, :],
                          m(wqk[128 * kt:128 * (kt + 1), :]))
        for kt in range(DK):
            nc.sync.dma_start(xt_all[:, kt, 0:SQ],
                              m(xT[128 * kt:128 * (kt + 1), 0:SQ]))
        for kt in range(DK):
            eng = nc.gpsimd if kt % 2 == 0 else nc.scalar
            eng.dma_start(wv_sb[:, kt, :],
                          m(wv[128 * kt:128 * (kt + 1), :]))
        for kt in range(DK):
            nc.sync.dma_start(
                xt_all[:, kt, SQ:],
                m(xT[128 * kt:128 * (kt + 1), SQ:]))
        for p4 in range(DQN):
            nc.gpsimd.dma_start(wo_sb[:, p4, :],
                                m(wo[128 * p4:128 * (p4 + 1), :]))

        def emit_qk_proj(blk):
            # q/k projections for the tokens of superblock blk only
            for dt in range(2 * DQN):
                pss = psum.tile([128, SQ], F32, tag="out", bufs=2,
                                name=f"pss_{blk}_{dt}")
                for kt in range(DK):
                    nc.tensor.matmul(
                        pss[:], m(wqk_sb[:, kt, 128 * dt:128 * (dt + 1)]),
                        m(xt_all[:, kt, blk * SQ:(blk + 1) * SQ]),
                        start=(kt == 0), stop=(kt == DK - 1))
                is_q = dt < DQN
                hp = dt % DQN
                dest = qT if is_q else kT
                nc.vector.tensor_scalar(
                    out=dest[:, hp, blk * SQ:(blk + 1) * SQ], in0=pss[:],
                    scalar1=bqk_sb[:, dt:dt + 1], scalar2=None,
                    op0=mybir.AluOpType.add)

        def emit_v_group(blk):
            # v projection for token tiles of one superblock (token-stationary)
            for tt in range(blk * ND, (blk + 1) * ND):
                psv = psum.tile([128, DQ], F32, tag="out", bufs=2,
                                name=f"psv_{tt}")
                for kt in range(DK):
                    nc.tensor.matmul(
                        psv[:], m(xt_all[:, kt, 128 * tt:128 * (tt + 1)]),
                        m(wv_sb[:, kt, :]),
                        start=(kt == 0), stop=(kt == DK - 1))
                va = v_aug[:, tt, :].rearrange("p (h c) -> p h c", h=HC)
                nc.vector.tensor_tensor(
                    va[:, :, 0:64], psv[:].rearrange("p (h c) -> p h c", h=HC),
                    bv_bc[:].rearrange("p (h c) -> p h c", h=HC),
                    op=mybir.AluOpType.add)
                nc.vector.tensor_copy(va[:, :, 64:65], ones_hc[:, :, None])

        # superblock-0 projections: 6 concurrent kt-accumulation chains
        # (2 double-wide tiles in the idle qk slots + 2 in out slots) so the
        # PE advances every chain as each wqk[kt]/xt[kt] DMA lands, instead
        # of one dt-chain stalling per tile arrival
        pss2 = [psum.tile([128, 2, SQ], F32, tag="qk", bufs=2,
                          name=f"pss0w_{dtp}") for dtp in range(2)]
        pss1 = [psum.tile([128, SQ], F32, tag="out", bufs=2,
                          name=f"pss0_{dt}") for dt in (4, 5)]
        for kt in range(DK):
            for dtp in range(2):
                for sub in range(2):
                    dt = 2 * dtp + sub
                    nc.tensor.matmul(
                        pss2[dtp][:, sub, :],
                        m(wqk_sb[:, kt, 128 * dt:128 * (dt + 1)]),
                        m(xt_all[:, kt, 0:SQ]),
                        start=(kt == 0), stop=(kt == DK - 1))
            for ii_, dt in enumerate((4, 5)):
                nc.tensor.matmul(
                    pss1[ii_], m(wqk_sb[:, kt, 128 * dt:128 * (dt + 1)]),
                    m(xt_all[:, kt, 0:SQ]),
                    start=(kt == 0), stop=(kt == DK - 1))

        def qk_epilogue(dt, src):
            is_q = dt < DQN
            hp = dt % DQN
            dest = qT if is_q else kT
            nc.vector.tensor_scalar(
                out=dest[:, hp, 0:SQ], in0=src,
                scalar1=bqk_sb[:, dt:dt + 1], scalar2=None,
                op0=mybir.AluOpType.add)

        for dtp in range(2):
            for sub in range(2):
                qk_epilogue(2 * dtp + sub, pss2[dtp][:, sub, :])
        for ii_, dt in enumerate((4, 5)):
            qk_epilogue(dt, pss1[ii_][:])
        for dt in (6, 7):
            pss = psum.tile([128, SQ], F32, tag="out", bufs=2,
                            name=f"pss0_{dt}")
            for kt in range(DK):
                nc.tensor.matmul(
                    pss[:], m(wqk_sb[:, kt, 128 * dt:128 * (dt + 1)]),
                    m(xt_all[:, kt, 0:SQ]),
                    start=(kt == 0), stop=(kt == DK - 1))
            qk_epilogue(dt, pss[:])

        # bv broadcast (needed by emit_v_group, off the startup critical path)
        bv_ps = psum.tile([128, DQ], F32, tag="out", bufs=2)
        nc.tensor.matmul(bv_ps[:], m(ones1b[:]), m(bv_row[:]),
                         start=True, stop=True)
        nc.scalar.copy(bv_bc[:], bv_ps[:])
        emit_v_group(0)

        for i in range(NSB):
            # ===== attention for superblock i ==============================
            NJ = ND * (i + 1)
            attnT = stream.tile([128, DQN, SQ], mm_dt, tag="attnT", bufs=3,
                                name=f"at_{i}")
            deferred = [None]

            def fire():
                if deferred[0] is not None:
                    deferred[0]()
                    deferred[0] = None

            for hp in range(DQN):
                # scores + exp for all key tiles of this head pair; probs
                # persist in SBUF so the two heads' PV chains can run as
                # separate single-bank psum accumulations (double-buffered)
                prb = []
                pv_tiles = []
                for j in range(NJ):
                    jj = j - ND * i
                    f0 = max(0, 128 * jj)
                    sc = psum.tile([128, 2, SQ], F32, tag="qk", bufs=2,
                                   name=f"sc_{i}_{hp}_{j}")
                    for hh in range(2):
                        p0, p1 = 64 * hh, 64 * hh + 64
                        nc.tensor.matmul(
                            sc[:, hh, f0:],
                            m(kT[p0:p1, hp, 128 * j:128 * (j + 1)]),
                            m(qT[p0:p1, hp, i * SQ + f0:(i + 1) * SQ]),
                            start=True, stop=True,
                            tile_position=(64 * hh, 0))
                    probs = stream.tile([128, 2, SQ], mm_dt, tag="probs",
                                        bufs=probs_bufs,
                                        name=f"pr_{i}_{hp}_{j}")
                    nc.scalar.activation(
                        probs[:, :, f0:], sc[:, :, f0:],
                        mybir.ActivationFunctionType.Exp)
                    if jj >= 0:
                        # zero the dead (key > query) triangle of the
                        # diagonal boundary subtile, post-exp (bf16 4x DVE)
                        for hh in range(2):
                            nc.vector.tensor_tensor(
                                probs[:, hh, f0:f0 + 128],
                                probs[:, hh, f0:f0 + 128], tri01[:],
                                op=mybir.AluOpType.mult)
                    prb.append((probs, f0))
                    if j == 0:
                        pv_tiles.append(psum.tile(
                            [65, SQ], F32, tag="pv", bufs=2,
                            name=f"pv_{i}_{hp}_0"))
                    if j == 1:
                        # previous head pair's tail fires here: its second
                        # head's PV stream fills the PE while this pair's
                        # exps run on ACT
                        fire()
                        if hp == 1 and i + 1 < NSB:
                            emit_v_group(i + 1)
                    if j >= 1:
                        pj = j - 1
                        pprobs, pf0 = prb[pj]
                        nc.tensor.matmul(
                            pv_tiles[0][:, pf0:],
                            m(v_aug[:, pj, 65 * (2 * hp):65 * (2 * hp) + 65]),
                            m(pprobs[:, 0, pf0:]),
                            start=(pj == 0), stop=False)
                pprobs, pf0 = prb[NJ - 1]
                nc.tensor.matmul(
                    pv_tiles[0][:, pf0:],
                    m(v_aug[:, NJ - 1, 65 * (2 * hp):65 * (2 * hp) + 65]),
                    m(pprobs[:, 0, pf0:]),
                    start=(NJ == 1), stop=True)

                def make_tail(prb=prb, pv_tiles=pv_tiles, hp=hp, at=attnT,
                              ii=i, NJ=NJ):
                    def emit():
                        pv_tiles.append(psum.tile(
                            [65, SQ], F32, tag="pv", bufs=2,
                            name=f"pv_{ii}_{hp}_1"))
                        h = 2 * hp + 1
                        for j in range(NJ):
                            pprobs, pf0 = prb[j]
                            nc.tensor.matmul(
                                pv_tiles[1][:, pf0:],
                                m(v_aug[:, j, 65 * h:65 * h + 65]),
                                m(pprobs[:, 1, pf0:]),
                                start=(j == 0), stop=(j == NJ - 1))
                        for hh in range(2):
                            pv1 = pv_tiles[hh]
                            # release copies: sumexp row (partition 64) +
                            # unnormalized features
                            se_r = stream.tile([65, SQ], BF16, tag="se",
                                               bufs=4,
                                               name=f"se_{ii}_{hp}_{hh}")
                            pv_sb = stream.tile([64, SQ], F32, tag="pvsb",
                                                bufs=4,
                                                name=f"pvs_{ii}_{hp}_{hh}")
                            nc.vector.tensor_copy(se_r[64:65, :],
                                                  pv1[64:65, :])
                            nc.vector.tensor_copy(pv_sb[:], pv1[0:64, :])
                            # normalization (broadcast sumexp via K=1 matmul
                            # at array row 64; the fast reciprocal needs base
                            # partition 0)
                            bc = psum.tile([64, SQ], F32, tag="out", bufs=2,
                                           name=f"bc_{ii}_{hp}_{hh}")
                            nc.tensor.matmul(bc[:], m(ones_hi[64:65, :]),
                                             m(se_r[64:65, :]),
                                             start=True, stop=True,
                                             tile_position=(64, 0))
                            bc_sb = stream.tile([64, SQ], F32, tag="bcs",
                                                bufs=3,
                                                name=f"bs_{ii}_{hp}_{hh}")
                            nc.vector.reciprocal_approx_fast(
                                out=bc_sb[:], in_=bc[:])
                            if hh == 0:
                                nc.vector.tensor_tensor(
                                    at[0:64, hp, :], pv_sb[:], bc_sb[:],
                                    op=mybir.AluOpType.mult)
                            else:
                                stage = stream.tile(
                                    [64, SQ], mm_dt, tag="stage", bufs=2,
                                    name=f"st_{ii}_{hp}")
                                nc.vector.tensor_tensor(
                                    stage[:], pv_sb[:], bc_sb[:],
                                    op=mybir.AluOpType.mult)
                                nc.sync.dma_start(at[64:128, hp, :],
                                                  stage[:])
                    return emit

                deferred[0] = make_tail()
            fire()

            # fill work for the next superblock (scheduler slots these into
            # PE gaps of the ACT-bound attention stretches)
            if i + 1 < NSB:
                emit_qk_proj(i + 1)

            # ===== output projection for superblock i ======================
            for mm_ in range(ND):
                tt = i * ND + mm_
                for nb in range(NOB):
                    # last superblock: attention is done, borrow the idle
                    # qk psum slots to double output-projection parallelism
                    ptag = "qk" if (i == NSB - 1 and nb == 1) else "out"
                    pos = psum.tile([128, NOUT], F32, tag=ptag, bufs=2,
                                    name=f"po_{tt}_{nb}")
                    for p4 in range(DQN):
                        nc.tensor.matmul(
                            pos[:],
                            m(attnT[:, p4, 128 * mm_:128 * (mm_ + 1)]),
                            m(wo_sb[:, p4, nb * NOUT:(nb + 1) * NOUT]),
                            start=(p4 == 0), stop=(p4 == DQN - 1))
                    osb = stream.tile([128, NOUT], F32, tag="osb", bufs=2,
                                      name=f"ob_{tt}_{nb}")
                    nc.vector.tensor_copy(osb[:], pos[:])
                    nc.sync.dma_start(
                        out[128 * tt:128 * (tt + 1),
                            nb * NOUT:(nb + 1) * NOUT], osb[:])

    nc.compile()
    return nc

B, S, D, H = 4, 2048, 1024, 16
N_CORES = 8

_CACHED = {}


def _make_core_inputs(x, Wq, bq, Wk, bk, Wv, bv, Wo):
    DQ = D // 2

    def cast(a):
        return np.ascontiguousarray(a).astype(ml_dtypes.bfloat16)

    xTs = [cast(x[b].T) for b in range(B)]
    in_maps = []
    for c in range(N_CORES):
        b, hf = c // 2, c % 2
        sl = slice(hf * DQ, (hf + 1) * DQ)
        in_maps.append({
            "xT": xTs[b],
            "wqk": cast(np.concatenate([0.125 * Wq[:, sl], Wk[:, sl]], axis=1)),
            "wv": cast(Wv[:, sl]),
            "wo": cast(Wo[sl, :]),
            "bqk": np.ascontiguousarray(
                np.concatenate([0.125 * bq[sl], bk[sl]])).astype(np.float32),
            "bv": np.ascontiguousarray(bv[sl]).astype(np.float32),
        })
    return in_maps


def kernel(x, Wq, bq, Wk, bk, Wv, bv, Wo, bo):
    import tempfile
    from concourse import bass_utils

    x = np.asarray(x, dtype=np.float32)
    Wq = np.asarray(Wq, dtype=np.float32)
    bq = np.asarray(bq, dtype=np.float32)
    Wk = np.asarray(Wk, dtype=np.float32)
    bk = np.asarray(bk, dtype=np.float32)
    Wv = np.asarray(Wv, dtype=np.float32)
    bv = np.asarray(bv, dtype=np.float32)
    Wo = np.asarray(Wo, dtype=np.float32)
    bo = np.asarray(bo, dtype=np.float32)

    if "nc" not in _CACHED:
        _CACHED["nc"] = build_core_program(S=S, D=D, HC=H // 2)
    nc = _CACHED["nc"]

    in_maps = _make_core_inputs(x, Wq, bq, Wk, bk, Wv, bv, Wo)
    res = bass_utils.run_bass_kernel_spmd(
        nc, in_maps, core_ids=list(range(N_CORES)),
        tmpdir=tempfile.mkdtemp(prefix="bass_attn_"))

    out = np.empty((B, S, D), dtype=np.float32)
    for b in range(B):
        out[b] = res.results[2 * b]["out"] + res.results[2 * b + 1]["out"] + bo
    return out


# revision 43
# speedup vs baseline: 1.0217x; 1.0217x over previous
"""Causal self-attention (B=4, S=2048, D=1024, H=16) on 8 TRN2 NeuronCores.

Sharding (tensor-parallel on heads + data-parallel on batch):
  core c -> batch c//2, head-half c%2 (8 of 16 heads).
  Wq/Wk/Wv column-split, Wo row-split; the two partial outputs per batch are
  summed on the host (+ bo), which is the row-parallel unshard.

Per-core Bass/Tile program (matmul operands bf16, psum/softmax fp32),
structured as a superblock pipeline so every engine always has ready work
(Tile's list scheduler dispatches ready instructions by emission priority):

  per 512-query superblock i:
    q/k projections for superblock i only (feature-major; bias+cast on DVE);
    superblock 0 runs 6 concurrent kt-accumulation chains (borrowing the
      idle qk psum slots) so the PE advances with each arriving DMA tile
      instead of stalling one chain per wqk[kt];
    v projection (token-major, per-head ones column produces sumexp in the
      PV matmul), emitted mid-attention as PE filler;
    attention per head pair: scores via two concurrent row-group matmuls
      into a combined [128, 2, 512] psum tile; ONE exp activation covers
      both heads (halves the 352-cycle ACT call overhead); the causal mask
      is applied AFTER exp as a multiplicative 0/1 bf16 mask on probs
      (off the scores->exp critical chain, 4x DVE mode);
    probs persist in SBUF so the two heads run as separate single-bank PV
      accumulations: head 0 pipelined one key tile behind exp, head 1 as a
      dense deferred stream that fires after the next pair's first exps are
      queued (fills PE while ACT works);
    normalization: sumexp row broadcast by a K=1 bf16 matmul at array row
      64, single-op fast reciprocal on the broadcast (base partition 0),
      DVE multiplies; pv psum released early via copies to SBUF;
    output projection per superblock, emitted as PE filler for the next
      superblock's attention.

461.7us baseline -> ~323us, rel err 3.8e-3 (tolerance 2e-2).
"""

from contextlib import ExitStack

import numpy as np
import ml_dtypes

import concourse.bass as bass
import concourse.bacc as bacc
import concourse.tile as tile
import concourse.mybir as mybir

F32 = mybir.dt.float32
F32R = mybir.dt.float32r
BF16 = mybir.dt.bfloat16

def build_core_program(S=2048, D=1024, HC=8, DH=64, SQ=512, mm_dt=BF16,
                       probs_bufs=17):
    """Build the per-core Bass program (SPMD: same program, different data).
    Host passes xT/wqk/wv/wo as bfloat16; 0.125 q-scale folded into Wq/bq."""
    DQ = HC * DH              # head-slice width (512)
    DK = D // 128             # contraction tiles for projections (8)
    DQN = DQ // 128           # head-pair tiles (4)
    NSB = S // SQ             # query superblocks (4)
    NTT = S // 128            # token tiles (16)
    NOUT = min(512, D)        # output-proj free width
    NOB = D // NOUT           # output-proj col blocks (2)
    ND = SQ // 128            # 128-token tiles per superblock (4)
    assert DQ % 128 == 0 and S % SQ == 0 and SQ % 128 == 0 and D % 128 == 0

    bf = mm_dt == BF16
    in_dt = BF16 if bf else F32

    def m(ap):
        return ap if bf else ap.bitcast(F32R)

    nc = bacc.Bacc("TRN2", target_bir_lowering=False, debug=False)

    xT = nc.dram_tensor("xT", [D, S], in_dt, kind="ExternalInput").ap()
    wqk = nc.dram_tensor("wqk", [D, 2 * DQ], in_dt, kind="ExternalInput").ap()
    wv = nc.dram_tensor("wv", [D, DQ], in_dt, kind="ExternalInput").ap()
    wo = nc.dram_tensor("wo", [DQ, D], in_dt, kind="ExternalInput").ap()
    bqk = nc.dram_tensor("bqk", [2 * DQ], F32, kind="ExternalInput").ap()
    bv = nc.dram_tensor("bv", [DQ], F32, kind="ExternalInput").ap()
    out = nc.dram_tensor("out", [S, D], F32, kind="ExternalOutput").ap()

    with tile.TileContext(nc) as tc, ExitStack() as ctx:
        ctx.enter_context(nc.allow_low_precision(
            reason="low-precision matmul operands; accumulation stays fp32"))
        const = ctx.enter_context(tc.tile_pool(name="const", bufs=1))
        big = ctx.enter_context(tc.tile_pool(name="big", bufs=1))
        stream = ctx.enter_context(tc.tile_pool(name="stream", bufs=1))
        psum = ctx.enter_context(tc.tile_pool(name="psum", bufs=1, space="PSUM"))

        # ---- constants ----
        # 0/1 causal mask for the diagonal boundary subtile (bf16, applied
        # multiplicatively to probs AFTER exp, off the scores->exp chain)
        tri_f = const.tile([128, 128], F32)
        nc.vector.memset(tri_f[:], 1.0)
        nc.gpsimd.affine_select(
            out=tri_f[:], in_=tri_f[:], compare_op=mybir.AluOpType.is_ge,
            fill=0.0, base=0, channel_multiplier=-1, pattern=[[1, 128]],
        )
        tri01 = const.tile([128, 128], BF16)
        nc.vector.tensor_copy(tri01[:], tri_f[:])
        ones_hc = const.tile([128, HC], F32)
        nc.vector.memset(ones_hc[:], 1.0)
        # ones row living on partition 64 (for K=1 matmuls whose moving
        # operand is the partition-64 sumexp row)
        ones_hi = const.tile([65, 64], BF16)
        nc.vector.memset(ones_hi[64:65, :], 1.0)

        # biases: bqk as [128, 2*DQN] (column t = dout tile t), bv broadcast
        bqk_sb = const.tile([128, 2 * DQN], F32)
        nc.sync.dma_start(bqk_sb[:], bqk.rearrange("(t p) -> p t", p=128))
        bv_rowf = const.tile([1, DQ], F32)
        nc.sync.dma_start(bv_rowf[:], bv.rearrange("(a d) -> a d", a=1))
        bv_row = const.tile([1, DQ], BF16)
        nc.vector.tensor_copy(bv_row[:], bv_rowf[:])
        ones1b = const.tile([1, 128], BF16)
        nc.vector.memset(ones1b[:], 1.0)
        bv_bc = const.tile([128, DQ], F32)

        # ---- big resident tensors ----
        kT = big.tile([128, DQN, S], mm_dt)     # [pair 2x64 rows, tokens]
        qT = big.tile([128, DQN, S], mm_dt)
        v_aug = big.tile([128, NTT, HC * 65], mm_dt)
        wqk_sb = big.tile([128, DK, 2 * DQ], mm_dt)
        wv_sb = big.tile([128, DK, DQ], mm_dt)
        wo_sb = big.tile([128, DQN, D], mm_dt)
        xt_all = big.tile([128, DK, S], mm_dt)

        # DMA priority order: wqk -> xt(sb0) -> wv -> xt(sb1..3) -> wo,
        # so superblock-0 projections and attention can start early
        for kt in range(DK):
            # alternate issue queues so the startup-critical wqk tiles
            # stream on two DMA queues in parallel
            eng = nc.gpsimd if kt % 2 == 0 else nc.scalar
            eng.dma_start(wqk_sb[:, kt, :],
                          m(wqk[128 * kt:128 * (kt + 1), :]))
        for kt in range(DK):
            nc.sync.dma_start(xt_all[:, kt, 0:SQ],
                              m(xT[128 * kt:128 * (kt + 1), 0:SQ]))
        for kt in range(DK):
            eng = nc.gpsimd if kt % 2 == 0 else nc.scalar
            eng.dma_start(wv_sb[:, kt, :],
                          m(wv[128 * kt:128 * (kt + 1), :]))
        for kt in range(DK):
            nc.sync.dma_start(
                xt_all[:, kt, SQ:],
                m(xT[128 * kt:128 * (kt + 1), SQ:]))
        for p4 in range(DQN):
            nc.gpsimd.dma_start(wo_sb[:, p4, :],
                                m(wo[128 * p4:128 * (p4 + 1), :]))

        def emit_qk_proj(blk):
            # q/k projections for the tokens of superblock blk only
            for dt in range(2 * DQN):
                pss = psum.tile([128, SQ], F32, tag="out", bufs=2,
                                name=f"pss_{blk}_{dt}")
                for kt in range(DK):
                    nc.tensor.matmul(
                        pss[:], m(wqk_sb[:, kt, 128 * dt:128 * (dt + 1)]),
                        m(xt_all[:, kt, blk * SQ:(blk + 1) * SQ]),
                        start=(kt == 0), stop=(kt == DK - 1))
                is_q = dt < DQN
                hp = dt % DQN
                dest = qT if is_q else kT
                nc.vector.tensor_scalar(
                    out=dest[:, hp, blk * SQ:(blk + 1) * SQ], in0=pss[:],
                    scalar1=bqk_sb[:, dt:dt + 1], scalar2=None,
                    op0=mybir.AluOpType.add)

        def emit_v_group(blk):
            # v projection for token tiles of one superblock (token-stationary)
            for tt in range(blk * ND, (blk + 1) * ND):
                psv = psum.tile([128, DQ], F32, tag="out", bufs=2,
                                name=f"psv_{tt}")
                for kt in range(DK):
                    nc.tensor.matmul(
                        psv[:], m(xt_all[:, kt, 128 * tt:128 * (tt + 1)]),
                        m(wv_sb[:, kt, :]),
                        start=(kt == 0), stop=(kt == DK - 1))
                va = v_aug[:, tt, :].rearrange("p (h c) -> p h c", h=HC)
                nc.vector.tensor_tensor(
                    va[:, :, 0:64], psv[:].rearrange("p (h c) -> p h c", h=HC),
                    bv_bc[:].rearrange("p (h c) -> p h c", h=HC),
                    op=mybir.AluOpType.add)
                nc.vector.tensor_copy(va[:, :, 64:65], ones_hc[:, :, None])

        # superblock-0 projections: 6 concurrent kt-accumulation chains
        # (2 double-wide tiles in the idle qk slots + 2 in out slots) so the
        # PE advances every chain as each wqk[kt]/xt[kt] DMA lands, instead
        # of one dt-chain stalling per tile arrival
        pss2 = [psum.tile([128, 2, SQ], F32, tag="qk", bufs=2,
                          name=f"pss0w_{dtp}") for dtp in range(2)]
        pss1 = [psum.tile([128, SQ], F32, tag="out", bufs=2,
                          name=f"pss0_{dt}") for dt in (4, 5)]
        for kt in range(DK):
            for dtp in range(2):
                for sub in range(2):
                    dt = 2 * dtp + sub
                    nc.tensor.matmul(
                        pss2[dtp][:, sub, :],
                        m(wqk_sb[:, kt, 128 * dt:128 * (dt + 1)]),
                        m(xt_all[:, kt, 0:SQ]),
                        start=(kt == 0), stop=(kt == DK - 1))
            for ii_, dt in enumerate((4, 5)):
                nc.tensor.matmul(
                    pss1[ii_], m(wqk_sb[:, kt, 128 * dt:128 * (dt + 1)]),
                    m(xt_all[:, kt, 0:SQ]),
                    start=(kt == 0), stop=(kt == DK - 1))

        def qk_epilogue(dt, src):
            is_q = dt < DQN
            hp = dt % DQN
            dest = qT if is_q else kT
            nc.vector.tensor_scalar(
                out=dest[:, hp, 0:SQ], in0=src,
                scalar1=bqk_sb[:, dt:dt + 1], scalar2=None,
                op0=mybir.AluOpType.add)

        for dtp in range(2):
            for sub in range(2):
                qk_epilogue(2 * dtp + sub, pss2[dtp][:, sub, :])
        for ii_, dt in enumerate((4, 5)):
            qk_epilogue(dt, pss1[ii_][:])
        for dt in (6, 7):
            pss = psum.tile([128, SQ], F32, tag="out", bufs=2,
                            name=f"pss0_{dt}")
            for kt in range(DK):
                nc.tensor.matmul(
                    pss[:], m(wqk_sb[:, kt, 128 * dt:128 * (dt + 1)]),
                    m(xt_all[:, kt, 0:SQ]),
                    start=(kt == 0), stop=(kt == DK - 1))
            qk_epilogue(dt, pss[:])

        # bv broadcast (needed by emit_v_group, off the startup critical path)
        bv_ps = psum.tile([128, DQ], F32, tag="out", bufs=2)
        nc.tensor.matmul(bv_ps[:], m(ones1b[:]), m(bv_row[:]),
                         start=True, stop=True)
        nc.scalar.copy(bv_bc[:], bv_ps[:])
        emit_v_group(0)

        for i in range(NSB):
            # ===== attention for superblock i ==============================
            NJ = ND * (i + 1)
            attnT = stream.tile([128, DQN, SQ], mm_dt, tag="attnT", bufs=2,
                                name=f"at_{i}")
            deferred = [None]

            def fire():
                if deferred[0] is not None:
                    deferred[0]()
                    deferred[0] = None

            for hp in range(DQN):
                # scores + exp for all key tiles of this head pair; probs
                # persist in SBUF so the two heads' PV chains can run as
                # separate single-bank psum accumulations (double-buffered)
                prb = []
                pv_tiles = []
                for j in range(NJ):
                    jj = j - ND * i
                    f0 = max(0, 128 * jj)
                    sc = psum.tile([128, 2, SQ], F32, tag="qk", bufs=2,
                                   name=f"sc_{i}_{hp}_{j}")
                    for hh in range(2):
                        p0, p1 = 64 * hh, 64 * hh + 64
                        nc.tensor.matmul(
                            sc[:, hh, f0:],
                            m(kT[p0:p1, hp, 128 * j:128 * (j + 1)]),
                            m(qT[p0:p1, hp, i * SQ + f0:(i + 1) * SQ]),
                            start=True, stop=True,
                            tile_position=(64 * hh, 0))
                    probs = stream.tile([128, 2, SQ], mm_dt, tag="probs",
                                        bufs=probs_bufs,
                                        name=f"pr_{i}_{hp}_{j}")
                    nc.scalar.activation(
                        probs[:, :, f0:], sc[:, :, f0:],
                        mybir.ActivationFunctionType.Exp)
                    if jj >= 0:
                        # zero the dead (key > query) triangle of the
                        # diagonal boundary subtile, post-exp (bf16 4x DVE)
                        for hh in range(2):
                            nc.vector.tensor_tensor(
                                probs[:, hh, f0:f0 + 128],
                                probs[:, hh, f0:f0 + 128], tri01[:],
                                op=mybir.AluOpType.mult)
                    prb.append((probs, f0))
                    if j == 0:
                        pv_tiles.append(psum.tile(
                            [65, SQ], F32, tag="pv", bufs=2,
                            name=f"pv_{i}_{hp}_0"))
                    if j == 1:
                        # previous head pair's tail fires here: its second
                        # head's PV stream fills the PE while this pair's
                        # exps run on ACT
                        fire()
                        if hp == 1 and i + 1 < NSB:
                            emit_v_group(i + 1)
                    if j >= 1:
                        pj = j - 1
                        pprobs, pf0 = prb[pj]
                        nc.tensor.matmul(
                            pv_tiles[0][:, pf0:],
                            m(v_aug[:, pj, 65 * (2 * hp):65 * (2 * hp) + 65]),
                            m(pprobs[:, 0, pf0:]),
                            start=(pj == 0), stop=False)
                pprobs, pf0 = prb[NJ - 1]
                nc.tensor.matmul(
                    pv_tiles[0][:, pf0:],
                    m(v_aug[:, NJ - 1, 65 * (2 * hp):65 * (2 * hp) + 65]),
                    m(pprobs[:, 0, pf0:]),
                    start=(NJ == 1), stop=True)

                def make_tail(prb=prb, pv_tiles=pv_tiles, hp=hp, at=attnT,
                              ii=i, NJ=NJ):
                    def emit():
                        pv_tiles.append(psum.tile(
                            [65, SQ], F32, tag="pv", bufs=2,
                            name=f"pv_{ii}_{hp}_1"))
                        h = 2 * hp + 1
                        for j in range(NJ):
                            pprobs, pf0 = prb[j]
                            nc.tensor.matmul(
                                pv_tiles[1][:, pf0:],
                                m(v_aug[:, j, 65 * h:65 * h + 65]),
                                m(pprobs[:, 1, pf0:]),
                                start=(j == 0), stop=(j == NJ - 1))
                        for hh in range(2):
                            pv1 = pv_tiles[hh]
                            # release copies: sumexp row (partition 64) +
                            # unnormalized features
                            se_r = stream.tile([65, SQ], BF16, tag="se",
                                               bufs=4,
                                               name=f"se_{ii}_{hp}_{hh}")
                            pv_sb = stream.tile([64, SQ], F32, tag="pvsb",
                                                bufs=4,
                                                name=f"pvs_{ii}_{hp}_{hh}")
                            nc.vector.tensor_copy(se_r[64:65, :],
                                                  pv1[64:65, :])
                            nc.vector.tensor_copy(pv_sb[:], pv1[0:64, :])
                            # normalization (broadcast sumexp via K=1 matmul
                            # at array row 64; the fast reciprocal needs base
                            # partition 0)
                            bc = psum.tile([64, SQ], F32, tag="out", bufs=2,
                                           name=f"bc_{ii}_{hp}_{hh}")
                            nc.tensor.matmul(bc[:], m(ones_hi[64:65, :]),
                                             m(se_r[64:65, :]),
                                             start=True, stop=True,
                                             tile_position=(64, 0))
                            bc_sb = stream.tile([64, SQ], F32, tag="bcs",
                                                bufs=3,
                                                name=f"bs_{ii}_{hp}_{hh}")
                            nc.vector.reciprocal_approx_fast(
                                out=bc_sb[:], in_=bc[:])
                            if hh == 0:
                                nc.vector.tensor_tensor(
                                    at[0:64, hp, :], pv_sb[:], bc_sb[:],
                                    op=mybir.AluOpType.mult)
                            else:
                                stage = stream.tile(
                                    [64, SQ], mm_dt, tag="stage", bufs=2,
                                    name=f"st_{ii}_{hp}")
                                nc.vector.tensor_tensor(
                                    stage[:], pv_sb[:], bc_sb[:],
                                    op=mybir.AluOpType.mult)
                                nc.sync.dma_start(at[64:128, hp, :],
                                                  stage[:])
                    return emit

                deferred[0] = make_tail()
            fire()

            # fill work for the next superblock (scheduler slots these into
            # PE gaps of the ACT-bound attention stretches)
            if i + 1 < NSB:
                emit_qk_proj(i + 1)

            # ===== output projection for superblock i ======================
            for mm_ in range(ND):
                tt = i * ND + mm_
                for nb in range(NOB):
                    # last superblock: attention is done, borrow the idle
                    # qk psum slots to double output-projection parallelism
                    ptag = "qk" if (i == NSB - 1 and nb == 1) else "out"
                    pos = psum.tile([128, NOUT], F32, tag=ptag, bufs=2,
                                    name=f"po_{tt}_{nb}")
                    for p4 in range(DQN):
                        nc.tensor.matmul(
                            pos[:],
                            m(attnT[:, p4, 128 * mm_:128 * (mm_ + 1)]),
                            m(wo_sb[:, p4, nb * NOUT:(nb + 1) * NOUT]),
                            start=(p4 == 0), stop=(p4 == DQN - 1))
                    osb = stream.tile([128, NOUT], F32, tag="osb", bufs=2,
                                      name=f"ob_{tt}_{nb}")
                    nc.vector.tensor_copy(osb[:], pos[:])
                    nc.sync.dma_start(
                        out[128 * tt:128 * (tt + 1),
                            nb * NOUT:(nb + 1) * NOUT], osb[:])

    nc.compile()
    return nc

B, S, D, H = 4, 2048, 1024, 16
N_CORES = 8

_CACHED = {}


def _make_core_inputs(x, Wq, bq, Wk, bk, Wv, bv, Wo):
    DQ = D // 2

    def cast(a):
        return np.ascontiguousarray(a).astype(ml_dtypes.bfloat16)

    xTs = [cast(x[b].T) for b in range(B)]
    in_maps = []
    for c in range(N_CORES):
        b, hf = c // 2, c % 2
        sl = slice(hf * DQ, (hf + 1) * DQ)
        in_maps.append({
            "xT": xTs[b],
            "wqk": cast(np.concatenate([0.125 * Wq[:, sl], Wk[:, sl]], axis=1)),
            "wv": cast(Wv[:, sl]),
            "wo": cast(Wo[sl, :]),
            "bqk": np.ascontiguousarray(
                np.concatenate([0.125 * bq[sl], bk[sl]])).astype(np.float32),
            "bv": np.ascontiguousarray(bv[sl]).astype(np.float32),
        })
    return in_maps


def kernel(x, Wq, bq, Wk, bk, Wv, bv, Wo, bo):
    import tempfile
    from concourse import bass_utils

    x = np.asarray(x, dtype=np.float32)
    Wq = np.asarray(Wq, dtype=np.float32)
    bq = np.asarray(bq, dtype=np.float32)
    Wk = np.asarray(Wk, dtype=np.float32)
    bk = np.asarray(bk, dtype=np.float32)
    Wv = np.asarray(Wv, dtype=np.float32)
    bv = np.asarray(bv, dtype=np.float32)
    Wo = np.asarray(Wo, dtype=np.float32)
    bo = np.asarray(bo, dtype=np.float32)

    if "nc" not in _CACHED:
        _CACHED["nc"] = build_core_program(S=S, D=D, HC=H // 2)
    nc = _CACHED["nc"]

    in_maps = _make_core_inputs(x, Wq, bq, Wk, bk, Wv, bv, Wo)
    res = bass_utils.run_bass_kernel_spmd(
        nc, in_maps, core_ids=list(range(N_CORES)),
        tmpdir=tempfile.mkdtemp(prefix="bass_attn_"))

    out = np.empty((B, S, D), dtype=np.float32)
    for b in range(B):
        out[b] = res.results[2 * b]["out"] + res.results[2 * b + 1]["out"] + bo
    return out


# revision 44
# speedup vs baseline: 1.0519x; 1.0295x over previous
"""Causal self-attention (B=4, S=2048, D=1024, H=16) on 8 TRN2 NeuronCores.

Sharding (tensor-parallel on heads + data-parallel on batch):
  core c -> batch c//2, head-half c%2 (8 of 16 heads).
  Wq/Wk/Wv column-split, Wo row-split; the two partial outputs per batch are
  summed on the host (+ bo), which is the row-parallel unshard.

Per-core Bass/Tile program (matmul operands bf16, psum/softmax fp32),
structured as a superblock pipeline so every engine always has ready work
(Tile's list scheduler dispatches ready instructions by emission priority):

  per 512-query superblock i:
    q/k projections for superblock i only (feature-major; bias+cast on DVE);
    superblock 0 runs 6 concurrent kt-accumulation chains (borrowing the
      idle qk psum slots) so the PE advances with each arriving DMA tile
      instead of stalling one chain per wqk[kt];
    v projection (token-major, per-head ones column produces sumexp in the
      PV matmul), emitted mid-attention as PE filler;
    attention per head pair: scores via two concurrent row-group matmuls
      into a combined [128, 2, 512] psum tile; ONE exp activation covers
      both heads (halves the 352-cycle ACT call overhead); the causal mask
      is applied AFTER exp as a multiplicative 0/1 bf16 mask on probs
      (off the scores->exp critical chain, 4x DVE mode);
    probs persist in SBUF so the two heads run as separate single-bank PV
      accumulations: head 0 pipelined one key tile behind exp, head 1 as a
      dense deferred stream that fires after the next pair's first exps are
      queued (fills PE while ACT works);
    normalization: sumexp row broadcast by a K=1 bf16 matmul at array row
      64, single-op fast reciprocal on the broadcast (base partition 0),
      DVE multiplies; pv psum released early via copies to SBUF;
    output projection per superblock, emitted as PE filler for the next
      superblock's attention.

461.7us baseline -> ~323us, rel err 3.8e-3 (tolerance 2e-2).
"""

from contextlib import ExitStack

import numpy as np
import ml_dtypes

import concourse.bass as bass
import concourse.bacc as bacc
import concourse.tile as tile
import concourse.mybir as mybir

F32 = mybir.dt.float32
F32R = mybir.dt.float32r
BF16 = mybir.dt.bfloat16

def build_core_program(S=2048, D=1024, HC=8, DH=64, SQ=512, mm_dt=BF16,
                       probs_bufs=17):
    """Build the per-core Bass program (SPMD: same program, different data).
    Host passes xT/wqk/wv/wo as bfloat16; 0.125 q-scale folded into Wq/bq."""
    DQ = HC * DH              # head-slice width (512)
    DK = D // 128             # contraction tiles for projections (8)
    DQN = DQ // 128           # head-pair tiles (4)
    NSB = S // SQ             # query superblocks (4)
    NTT = S // 128            # token tiles (16)
    NOUT = min(512, D)        # output-proj free width
    NOB = D // NOUT           # output-proj col blocks (2)
    ND = SQ // 128            # 128-token tiles per superblock (4)
    assert DQ % 128 == 0 and S % SQ == 0 and SQ % 128 == 0 and D % 128 == 0

    bf = mm_dt == BF16
    in_dt = BF16 if bf else F32

    def m(ap):
        return ap if bf else ap.bitcast(F32R)

    nc = bacc.Bacc("TRN2", target_bir_lowering=False, debug=False)

    xT = nc.dram_tensor("xT", [D, S], in_dt, kind="ExternalInput").ap()
    wqk = nc.dram_tensor("wqk", [D, 2 * DQ], in_dt, kind="ExternalInput").ap()
    wv = nc.dram_tensor("wv", [D, DQ], in_dt, kind="ExternalInput").ap()
    wo = nc.dram_tensor("wo", [DQ, D], in_dt, kind="ExternalInput").ap()
    bqk = nc.dram_tensor("bqk", [2 * DQ], F32, kind="ExternalInput").ap()
    bv = nc.dram_tensor("bv", [DQ], F32, kind="ExternalInput").ap()
    out = nc.dram_tensor("out", [S, D], F32, kind="ExternalOutput").ap()

    with tile.TileContext(nc) as tc, ExitStack() as ctx:
        ctx.enter_context(nc.allow_low_precision(
            reason="low-precision matmul operands; accumulation stays fp32"))
        const = ctx.enter_context(tc.tile_pool(name="const", bufs=1))
        big = ctx.enter_context(tc.tile_pool(name="big", bufs=1))
        stream = ctx.enter_context(tc.tile_pool(name="stream", bufs=1))
        psum = ctx.enter_context(tc.tile_pool(name="psum", bufs=1, space="PSUM"))

        # ---- constants ----
        # 0/1 causal mask for the diagonal boundary subtile (bf16, applied
        # multiplicatively to probs AFTER exp, off the scores->exp chain)
        tri_f = const.tile([128, 128], F32)
        nc.vector.memset(tri_f[:], 1.0)
        nc.gpsimd.affine_select(
            out=tri_f[:], in_=tri_f[:], compare_op=mybir.AluOpType.is_ge,
            fill=0.0, base=0, channel_multiplier=-1, pattern=[[1, 128]],
        )
        tri01 = const.tile([128, 128], BF16)
        nc.vector.tensor_copy(tri01[:], tri_f[:])
        ones_hc = const.tile([128, HC], F32)
        nc.vector.memset(ones_hc[:], 1.0)
        # ones row living on partition 64 (for K=1 matmuls whose moving
        # operand is the partition-64 sumexp row)
        ones_hi = const.tile([65, 64], BF16)
        nc.vector.memset(ones_hi[64:65, :], 1.0)

        # biases: bqk as [128, 2*DQN] (column t = dout tile t), bv broadcast
        bqk_sb = const.tile([128, 2 * DQN], F32)
        nc.sync.dma_start(bqk_sb[:], bqk.rearrange("(t p) -> p t", p=128))
        bv_rowf = const.tile([1, DQ], F32)
        nc.sync.dma_start(bv_rowf[:], bv.rearrange("(a d) -> a d", a=1))
        bv_row = const.tile([1, DQ], BF16)
        nc.vector.tensor_copy(bv_row[:], bv_rowf[:])
        ones1b = const.tile([1, 128], BF16)
        nc.vector.memset(ones1b[:], 1.0)
        bv_bc = const.tile([128, DQ], F32)

        # ---- big resident tensors ----
        kT = big.tile([128, DQN, S], mm_dt)     # [pair 2x64 rows, tokens]
        qT = big.tile([128, DQN, S], mm_dt)
        v_aug = big.tile([128, NTT, HC * 65], mm_dt)
        wqk_sb = big.tile([128, DK, 2 * DQ], mm_dt)
        wv_sb = big.tile([128, DK, DQ], mm_dt)
        wo_sb = big.tile([128, DQN, D], mm_dt)
        xt_all = big.tile([128, DK, S], mm_dt)

        # DMA priority order: wqk -> xt(sb0) -> wv -> xt(sb1..3) -> wo,
        # so superblock-0 projections and attention can start early
        for kt in range(DK):
            # alternate issue queues so the startup-critical wqk tiles
            # stream on two DMA queues in parallel
            eng = nc.gpsimd if kt % 2 == 0 else nc.scalar
            eng.dma_start(wqk_sb[:, kt, :],
                          m(wqk[128 * kt:128 * (kt + 1), :]))
        for kt in range(DK):
            nc.sync.dma_start(xt_all[:, kt, 0:SQ],
                              m(xT[128 * kt:128 * (kt + 1), 0:SQ]))
        for kt in range(DK):
            eng = nc.gpsimd if kt % 2 == 0 else nc.scalar
            eng.dma_start(wv_sb[:, kt, :],
                          m(wv[128 * kt:128 * (kt + 1), :]))
        for kt in range(DK):
            nc.sync.dma_start(
                xt_all[:, kt, SQ:],
                m(xT[128 * kt:128 * (kt + 1), SQ:]))
        for p4 in range(DQN):
            nc.gpsimd.dma_start(wo_sb[:, p4, :],
                                m(wo[128 * p4:128 * (p4 + 1), :]))

        def emit_qk_proj(blk):
            # q/k projections for the tokens of superblock blk only
            for dt in range(2 * DQN):
                pss = psum.tile([128, SQ], F32, tag="out", bufs=2,
                                name=f"pss_{blk}_{dt}")
                for kt in range(DK):
                    nc.tensor.matmul(
                        pss[:], m(wqk_sb[:, kt, 128 * dt:128 * (dt + 1)]),
                        m(xt_all[:, kt, blk * SQ:(blk + 1) * SQ]),
                        start=(kt == 0), stop=(kt == DK - 1))
                is_q = dt < DQN
                hp = dt % DQN
                dest = qT if is_q else kT
                nc.vector.tensor_scalar(
                    out=dest[:, hp, blk * SQ:(blk + 1) * SQ], in0=pss[:],
                    scalar1=bqk_sb[:, dt:dt + 1], scalar2=None,
                    op0=mybir.AluOpType.add)

        def emit_v_group(blk):
            # v projection for token tiles of one superblock (token-stationary)
            for tt in range(blk * ND, (blk + 1) * ND):
                psv = psum.tile([128, DQ], F32, tag="out", bufs=2,
                                name=f"psv_{tt}")
                for kt in range(DK):
                    nc.tensor.matmul(
                        psv[:], m(xt_all[:, kt, 128 * tt:128 * (tt + 1)]),
                        m(wv_sb[:, kt, :]),
                        start=(kt == 0), stop=(kt == DK - 1))
                va = v_aug[:, tt, :].rearrange("p (h c) -> p h c", h=HC)
                nc.vector.tensor_tensor(
                    va[:, :, 0:64], psv[:].rearrange("p (h c) -> p h c", h=HC),
                    bv_bc[:].rearrange("p (h c) -> p h c", h=HC),
                    op=mybir.AluOpType.add)
                nc.vector.tensor_copy(va[:, :, 64:65], ones_hc[:, :, None])

        # superblock-0 projections: 6 concurrent kt-accumulation chains
        # (2 double-wide tiles in the idle qk slots + 2 in out slots) so the
        # PE advances every chain as each wqk[kt]/xt[kt] DMA lands, instead
        # of one dt-chain stalling per tile arrival
        pss2 = [psum.tile([128, 2, SQ], F32, tag="qk", bufs=2,
                          name=f"pss0w_{dtp}") for dtp in range(2)]
        pss1 = [psum.tile([128, SQ], F32, tag="out", bufs=2,
                          name=f"pss0_{dt}") for dt in (4, 5)]
        for kt in range(DK):
            for dtp in range(2):
                for sub in range(2):
                    dt = 2 * dtp + sub
                    nc.tensor.matmul(
                        pss2[dtp][:, sub, :],
                        m(wqk_sb[:, kt, 128 * dt:128 * (dt + 1)]),
                        m(xt_all[:, kt, 0:SQ]),
                        start=(kt == 0), stop=(kt == DK - 1))
            for ii_, dt in enumerate((4, 5)):
                nc.tensor.matmul(
                    pss1[ii_], m(wqk_sb[:, kt, 128 * dt:128 * (dt + 1)]),
                    m(xt_all[:, kt, 0:SQ]),
                    start=(kt == 0), stop=(kt == DK - 1))

        def qk_epilogue(dt, src):
            is_q = dt < DQN
            hp = dt % DQN
            dest = qT if is_q else kT
            nc.vector.tensor_scalar(
                out=dest[:, hp, 0:SQ], in0=src,
                scalar1=bqk_sb[:, dt:dt + 1], scalar2=None,
                op0=mybir.AluOpType.add)

        for dtp in range(2):
            for sub in range(2):
                qk_epilogue(2 * dtp + sub, pss2[dtp][:, sub, :])
        for ii_, dt in enumerate((4, 5)):
            qk_epilogue(dt, pss1[ii_][:])
        for dt in (6, 7):
            pss = psum.tile([128, SQ], F32, tag="out", bufs=2,
                            name=f"pss0_{dt}")
            for kt in range(DK):
                nc.tensor.matmul(
                    pss[:], m(wqk_sb[:, kt, 128 * dt:128 * (dt + 1)]),
                    m(xt_all[:, kt, 0:SQ]),
                    start=(kt == 0), stop=(kt == DK - 1))
            qk_epilogue(dt, pss[:])

        # bv broadcast (needed by emit_v_group, off the startup critical path)
        bv_ps = psum.tile([128, DQ], F32, tag="out", bufs=2)
        nc.tensor.matmul(bv_ps[:], m(ones1b[:]), m(bv_row[:]),
                         start=True, stop=True)
        nc.scalar.copy(bv_bc[:], bv_ps[:])
        emit_v_group(0)

        for i in range(NSB):
            # ===== attention for superblock i ==============================
            NJ = ND * (i + 1)
            attnT = stream.tile([128, DQN, SQ], mm_dt, tag="attnT", bufs=2,
                                name=f"at_{i}")
            deferred = [None]

            def fire():
                if deferred[0] is not None:
                    deferred[0]()
                    deferred[0] = None

            for hp in range(DQN):
                # scores + exp for all key tiles of this head pair; probs
                # persist in SBUF so the two heads' PV chains can run as
                # separate single-bank psum accumulations (double-buffered)
                prb = []
                pv_tiles = []
                for j in range(NJ):
                    jj = j - ND * i
                    f0 = max(0, 128 * jj)
                    sc = psum.tile([128, 2, SQ], F32, tag="qk", bufs=2,
                                   name=f"sc_{i}_{hp}_{j}")
                    for hh in range(2):
                        p0, p1 = 64 * hh, 64 * hh + 64
                        nc.tensor.matmul(
                            sc[:, hh, f0:],
                            m(kT[p0:p1, hp, 128 * j:128 * (j + 1)]),
                            m(qT[p0:p1, hp, i * SQ + f0:(i + 1) * SQ]),
                            start=True, stop=True,
                            tile_position=(64 * hh, 0))
                    probs = stream.tile([128, 2, SQ], mm_dt, tag="probs",
                                        bufs=probs_bufs,
                                        name=f"pr_{i}_{hp}_{j}")
                    nc.scalar.activation(
                        probs[:, :, f0:], sc[:, :, f0:],
                        mybir.ActivationFunctionType.Exp)
                    if jj >= 0:
                        # zero the dead (key > query) triangle of the
                        # diagonal boundary subtile, post-exp (bf16 4x DVE)
                        for hh in range(2):
                            nc.vector.tensor_tensor(
                                probs[:, hh, f0:f0 + 128],
                                probs[:, hh, f0:f0 + 128], tri01[:],
                                op=mybir.AluOpType.mult)
                    prb.append((probs, f0))
                    if j == 0:
                        pv_tiles.append(psum.tile(
                            [65, SQ], F32, tag="pv", bufs=2,
                            name=f"pv_{i}_{hp}_0"))
                    if j == 1:
                        # previous head pair's tail fires here: its second
                        # head's PV stream fills the PE while this pair's
                        # exps run on ACT
                        fire()
                        if hp == 1 and i + 1 < NSB:
                            emit_v_group(i + 1)
                        if hp == 2 and i + 1 < NSB:
                            emit_qk_proj(i + 1)
                    if j >= 1:
                        pj = j - 1
                        pprobs, pf0 = prb[pj]
                        nc.tensor.matmul(
                            pv_tiles[0][:, pf0:],
                            m(v_aug[:, pj, 65 * (2 * hp):65 * (2 * hp) + 65]),
                            m(pprobs[:, 0, pf0:]),
                            start=(pj == 0), stop=False)
                pprobs, pf0 = prb[NJ - 1]
                nc.tensor.matmul(
                    pv_tiles[0][:, pf0:],
                    m(v_aug[:, NJ - 1, 65 * (2 * hp):65 * (2 * hp) + 65]),
                    m(pprobs[:, 0, pf0:]),
                    start=(NJ == 1), stop=True)

                def make_tail(prb=prb, pv_tiles=pv_tiles, hp=hp, at=attnT,
                              ii=i, NJ=NJ):
                    def emit():
                        pv_tiles.append(psum.tile(
                            [65, SQ], F32, tag="pv", bufs=2,
                            name=f"pv_{ii}_{hp}_1"))
                        h = 2 * hp + 1
                        for j in range(NJ):
                            pprobs, pf0 = prb[j]
                            nc.tensor.matmul(
                                pv_tiles[1][:, pf0:],
                                m(v_aug[:, j, 65 * h:65 * h + 65]),
                                m(pprobs[:, 1, pf0:]),
                                start=(j == 0), stop=(j == NJ - 1))
                        for hh in range(2):
                            pv1 = pv_tiles[hh]
                            # release copies: sumexp row (partition 64) +
                            # unnormalized features
                            se_r = stream.tile([65, SQ], BF16, tag="se",
                                               bufs=4,
                                               name=f"se_{ii}_{hp}_{hh}")
                            pv_sb = stream.tile([64, SQ], F32, tag="pvsb",
                                                bufs=4,
                                                name=f"pvs_{ii}_{hp}_{hh}")
                            nc.vector.tensor_copy(se_r[64:65, :],
                                                  pv1[64:65, :])
                            nc.vector.tensor_copy(pv_sb[:], pv1[0:64, :])
                            # normalization (broadcast sumexp via K=1 matmul
                            # at array row 64; the fast reciprocal needs base
                            # partition 0)
                            bc = psum.tile([64, SQ], F32, tag="out", bufs=2,
                                           name=f"bc_{ii}_{hp}_{hh}")
                            nc.tensor.matmul(bc[:], m(ones_hi[64:65, :]),
                                             m(se_r[64:65, :]),
                                             start=True, stop=True,
                                             tile_position=(64, 0))
                            bc_sb = stream.tile([64, SQ], F32, tag="bcs",
                                                bufs=3,
                                                name=f"bs_{ii}_{hp}_{hh}")
                            nc.vector.reciprocal_approx_fast(
                                out=bc_sb[:], in_=bc[:])
                            if hh == 0:
                                nc.vector.tensor_tensor(
                                    at[0:64, hp, :], pv_sb[:], bc_sb[:],
                                    op=mybir.AluOpType.mult)
                            else:
                                stage = stream.tile(
                                    [64, SQ], mm_dt, tag="stage", bufs=2,
                                    name=f"st_{ii}_{hp}")
                                nc.vector.tensor_tensor(
                                    stage[:], pv_sb[:], bc_sb[:],
                                    op=mybir.AluOpType.mult)
                                nc.sync.dma_start(at[64:128, hp, :],
                                                  stage[:])
                    return emit

                deferred[0] = make_tail()
            fire()


            # ===== output projection for superblock i ======================
            for mm_ in range(ND):
                tt = i * ND + mm_
                for nb in range(NOB):
                    # last superblock: attention is done, borrow the idle
                    # qk psum slots to double output-projection parallelism
                    ptag = "qk" if (i == NSB - 1 and nb == 1) else "out"
                    pos = psum.tile([128, NOUT], F32, tag=ptag, bufs=2,
                                    name=f"po_{tt}_{nb}")
                    for p4 in range(DQN):
                        nc.tensor.matmul(
                            pos[:],
                            m(attnT[:, p4, 128 * mm_:128 * (mm_ + 1)]),
                            m(wo_sb[:, p4, nb * NOUT:(nb + 1) * NOUT]),
                            start=(p4 == 0), stop=(p4 == DQN - 1))
                    osb = stream.tile([128, NOUT], F32, tag="osb", bufs=2,
                                      name=f"ob_{tt}_{nb}")
                    nc.vector.tensor_copy(osb[:], pos[:])
                    nc.sync.dma_start(
                        out[128 * tt:128 * (tt + 1),
                            nb * NOUT:(nb + 1) * NOUT], osb[:])

    nc.compile()
    return nc

B, S, D, H = 4, 2048, 1024, 16
N_CORES = 8

_CACHED = {}


def _make_core_inputs(x, Wq, bq, Wk, bk, Wv, bv, Wo):
    DQ = D // 2

    def cast(a):
        return np.ascontiguousarray(a).astype(ml_dtypes.bfloat16)

    xTs = [cast(x[b].T) for b in range(B)]
    in_maps = []
    for c in range(N_CORES):
        b, hf = c // 2, c % 2
        sl = slice(hf * DQ, (hf + 1) * DQ)
        in_maps.append({
            "xT": xTs[b],
            "wqk": cast(np.concatenate([0.125 * Wq[:, sl], Wk[:, sl]], axis=1)),
            "wv": cast(Wv[:, sl]),
            "wo": cast(Wo[sl, :]),
            "bqk": np.ascontiguousarray(
                np.concatenate([0.125 * bq[sl], bk[sl]])).astype(np.float32),
            "bv": np.ascontiguousarray(bv[sl]).astype(np.float32),
        })
    return in_maps


def kernel(x, Wq, bq, Wk, bk, Wv, bv, Wo, bo):
    import tempfile
    from concourse import bass_utils

    x = np.asarray(x, dtype=np.float32)
    Wq = np.asarray(Wq, dtype=np.float32)
    bq = np.asarray(bq, dtype=np.float32)
    Wk = np.asarray(Wk, dtype=np.float32)
    bk = np.asarray(bk, dtype=np.float32)
    Wv = np.asarray(Wv, dtype=np.float32)
    bv = np.asarray(bv, dtype=np.float32)
    Wo = np.asarray(Wo, dtype=np.float32)
    bo = np.asarray(bo, dtype=np.float32)

    if "nc" not in _CACHED:
        _CACHED["nc"] = build_core_program(S=S, D=D, HC=H // 2)
    nc = _CACHED["nc"]

    in_maps = _make_core_inputs(x, Wq, bq, Wk, bk, Wv, bv, Wo)
    res = bass_utils.run_bass_kernel_spmd(
        nc, in_maps, core_ids=list(range(N_CORES)),
        tmpdir=tempfile.mkdtemp(prefix="bass_attn_"))

    out = np.empty((B, S, D), dtype=np.float32)
    for b in range(B):
        out[b] = res.results[2 * b]["out"] + res.results[2 * b + 1]["out"] + bo
    return out
